# Initial kernel scaffold
#
"""DCNv4 block (cv1 1x1 -> offset/mask proj -> deformable bilinear sampling
-> cv2 1x1 -> BN -> SiLU) as a Bass/Tile kernel for Trainium2.

Strategy
--------
Data-parallel over batch: each of the 8 NeuronCores processes one image.

The deformable sampling is reformulated gather-free: with |off| < 1 the
bilinear sample of kernel point k at (h+kh+off_h, w+kw+off_w) equals
  sum_{i,j in {-1,0,1}} tent(off_h - i) * tent(off_w - j) * V[h+kh+i, w+kw+j]
with tent(t) = max(0, 1-|t|).  Merging all 9 kernel points over absolute
displacements e=(eh,ew) in [-2,2]^2 gives 25 "taps":
  out[p,g,:] = sum_e A_e[p,g] * Vpad[p+e, g, :]
  A_e[p,g]   = sum_k mask_k * tent(off_h - (eh-kh)) * tent(off_w - (ew-kw))
Out-of-image corners are handled exactly by zero-padding Vpad (the reference
drops those corners).

Engine mapping / schedule:
 - PE: cv1 / om / cv2 matmuls (f32r), A^T transposes, and the 25-term tap
   accumulation as identity-weight matmuls into PSUM.  om/cv1 matmuls are
   emitted just-in-time (one quarter ahead) so the deep PE exec queue always
   holds satisfied-dep work and the p-state stays ramped.
 - DVE: tent slot-1 builds, mask multiply and the 9 tent products (merged
   over i/j via stride-0 broadcast dims), 9 scatter-adds into A^T, and most
   tap products as single [128, 2, 1024] ops covering both channel tiles
   (the A map broadcast over vt with a stride-0 dim, vpad holding vt as a
   real dim).
 - GPSIMD: A^T zeroing for early chunks, pad-ring memsets, and for a
   balanced share of taps the vt0 product (TensorTensor; the cost of DVE's
   remaining vt1 single then balances the two engines).
 - ACT: om bf16 copies (the mask slice doubles as the bf16 mask, no
   separate cast), tent relus, cv1-bias copies (bias rides Identity), A^T
   zeroing late chunks, atile/usb copies, BN+SiLU epilogue.
 - Schedule: A-build for chunks 2q+2, 2q+3 is emitted BEFORE tap quarter q
   (software pipelining), so neither DVE nor ACT ever wait behind a
   quarter's tap/idn stream; separate tile pools for Pool- vs DVE-assigned
   taps prevent buffer-rotation convoys.
"""

import sys
import numpy as np

if "/opt/trn_rl_repo" not in sys.path:
    sys.path.insert(0, "/opt/trn_rl_repo")

import ml_dtypes

B, C1, C2, H, W = 8, 256, 256, 64, 64
C = 256
G = 16
Cg = 16
K = 9
HW = H * W           # 4096
PW = W + 4           # 68
PH = H + 4
BN_EPS = 1e-5
TPAD = 32            # taps padded to 32 so (t, g) blocks are 128-aligned
OMW = 448            # om channels padded 432 -> 448

_cache = {}


def _v_perm():
    perm = []
    for vt in range(2):
        for j in range(128):
            g = j // 8
            c = vt * 8 + (j % 8)
            perm.append(g * Cg + c)
    return np.array(perm, np.int64)


def _om_perm():
    rows = np.zeros(432, np.int64)
    for r in range(144):
        k, g = r // 16, r % 16
        rows[r] = g * 27 + 2 * k            # off_h (dh)
        rows[144 + r] = g * 27 + 2 * k + 1  # off_w (dw)
        rows[288 + r] = g * 27 + 18 + k     # mask
    return rows


def _split_multiwait(nc, mybir, max_waits=1):
    """walrus in this container rejects >1 sem wait on one instruction;
    split extras onto preceding same-engine NoOps (equivalent ordering)."""
    for f in nc.m.functions:
        for bb in f.blocks:
            out = []
            for inst in bb.instructions:
                si = inst.sync_info
                if si is not None and len(si.on_wait) > max_waits:
                    waits = list(si.on_wait)
                    for w in waits[:-max_waits]:
                        nop = mybir.InstNoOp(
                            name=f"I-nopw{nc.next_id()}", ins=[], outs=[])
                        nop.engine = inst.engine
                        nop.sync_info = mybir.SyncInfo(on_wait=[w], on_update=[])
                        nc.register_instruction(nop)
                        out.append(nop)
                    si.on_wait = waits[-max_waits:]
                out.append(inst)
            bb.instructions = out


def _build_nc(phase=99):
    import concourse.bass as bass
    import concourse.mybir as mybir
    import concourse.tile as tile

    f32 = mybir.dt.float32
    f32r = mybir.dt.float32r
    bf16 = mybir.dt.bfloat16
    ALU = mybir.AluOpType
    ACTF = mybir.ActivationFunctionType

    nc = bass.Bass()

    x_d = nc.dram_tensor("x", [C1, HW], f32r, kind="ExternalInput")
    wt1_d = nc.dram_tensor("wt1", [C1, 256], f32r, kind="ExternalInput")
    wtom_d = nc.dram_tensor("wtom", [C1, OMW], f32r, kind="ExternalInput")
    wt2_d = nc.dram_tensor("wt2", [C, C2], bf16, kind="ExternalInput")
    b1c_d = nc.dram_tensor("b1c", [C, 1], f32, kind="ExternalInput")
    b2_d = nc.dram_tensor("b2", [C2, 1], f32, kind="ExternalInput")
    bom_d = nc.dram_tensor("bom", [1, OMW], f32r, kind="ExternalInput")
    idn_d = nc.dram_tensor("idn", [128, 128], bf16, kind="ExternalInput")
    ones_d = nc.dram_tensor("onesrow", [1, 128], f32r, kind="ExternalInput")
    y_d = nc.dram_tensor("y", [C2, HW], f32, kind="ExternalOutput")

    cpt = 4
    n_chunk = 8
    QPIX = 1024
    EHS = (-2, -1, 0, 1, 2)

    with tile.TileContext(nc) as tc:
        with tc.tile_pool(name="persist", bufs=1) as persist:

            # ---- persistent tiles ----
            wt1s = [persist.tile([128, 256], f32r, name=f"wt1_{i}") for i in range(2)]
            wtoms = [persist.tile([128, OMW], f32r, name=f"wtom_{i}") for i in range(2)]
            wt2s = [persist.tile([128, 256], bf16, name=f"wt2_{i}") for i in range(2)]
            bom1 = persist.tile([1, OMW], f32r, name="bom1")
            b1cols = [persist.tile([128, 1], f32, name=f"b1c_{i}") for i in range(2)]
            b2s = [persist.tile([128, 1], f32, name=f"b2_{i}") for i in range(2)]
            ones = persist.tile([1, 128], f32r, name="ones")
            idn = persist.tile([128, 128], bf16, name="idn")
            vpad = persist.tile([128, 2, PH, PW], bf16, name="vpad")
            atile = [persist.tile([128, HW], bf16, name=f"atile_{i}") for i in range(4)]
            usb = [persist.tile([128, HW], bf16, name=f"usb_{v}") for v in range(2)]

            # zero the pad ring of Vpad (interior written by cv1)
            for vt in range(2):
                nc.gpsimd.memset(vpad[:, vt, 0:2, :], 0.0)
                nc.gpsimd.memset(vpad[:, vt, PH - 2:PH, :], 0.0)
                nc.gpsimd.memset(vpad[:, vt, 2:PH - 2, 0:2], 0.0)
                nc.gpsimd.memset(vpad[:, vt, 2:PH - 2, PW - 4:PW], 0.0)

            # ---- interleaved: om chunks + A-build + tap quarters.  om/cv1
            # matmuls stay just-in-time per chunk so the deep PE exec queue
            # always holds satisfied-dep work (keeps the PE p-state ramped)
            with tc.tile_pool(name="build", bufs=1) as bpool, \
                 tc.tile_pool(name="psA", bufs=2, space="PSUM") as psA, \
                 tc.tile_pool(name="ombuf", bufs=4) as ombuf, \
                 tc.tile_pool(name="tbuf", bufs=2) as tbuf, \
                 tc.tile_pool(name="atbuf", bufs=2) as atbuf, \
                 tc.tile_pool(name="trps", bufs=2, space="PSUM") as trps, \
                 tc.tile_pool(name="abcp", bufs=5) as abcp, \
                 tc.tile_pool(name="abcq", bufs=5) as abcq, \
                 tc.tile_pool(name="prodp", bufs=3) as prodp, \
                 tc.tile_pool(name="prodq", bufs=4) as prodq:

                scr = bpool.tile([128, 512], bf16, name="pewarm")
                nc.vector.memset(scr, 0.0)
                for wub in range(6):
                    wps = psA.tile([128, 512], f32, name="omm")
                    nc.tensor.matmul(wps, lhsT=scr[:, 0:128], rhs=scr[:, :],
                                     start=True, stop=True)

                xs = [bpool.tile([128, HW], f32r, name=f"xs_{i}") for i in range(2)]
                for i in range(2):
                    nc.sync.dma_start(out=xs[i][:, 0:512],
                                      in_=x_d[i * 128:(i + 1) * 128, 0:512])
                for i in range(2):
                    nc.sync.dma_start(out=wtoms[i],
                                      in_=wtom_d[i * 128:(i + 1) * 128, :])
                nc.sync.dma_start(out=bom1, in_=bom_d[:, :])
                nc.sync.dma_start(out=ones, in_=ones_d[:, :])
                for q4 in range(4):
                    lo = 512 if q4 == 0 else q4 * 1024
                    for i in range(2):
                        nc.sync.dma_start(
                            out=xs[i][:, lo:(q4 + 1) * 1024],
                            in_=x_d[i * 128:(i + 1) * 128, lo:(q4 + 1) * 1024])
                nc.sync.dma_start(out=idn, in_=idn_d[:, :])
                for i in range(2):
                    nc.sync.dma_start(out=wt1s[i], in_=wt1_d[i * 128:(i + 1) * 128, :])
                    nc.sync.dma_start(out=wt2s[i], in_=wt2_d[i * 128:(i + 1) * 128, :])
                    nc.sync.dma_start(out=b1cols[i], in_=b1c_d[i * 128:(i + 1) * 128, :])
                    nc.sync.dma_start(out=b2s[i], in_=b2_d[i * 128:(i + 1) * 128, :])

                def emit_vblock(nt):
                    # cv1 rows nt*8..nt*8+8; bias rides the activation copy
                    for mt in range(2):
                        ps = psA.tile([128, 512], f32, name="omm")
                        for kt in range(2):
                            nc.tensor.matmul(
                                ps, lhsT=wt1s[kt][:, mt * 128:(mt + 1) * 128],
                                rhs=xs[kt][:, nt * 512:(nt + 1) * 512],
                                start=(kt == 0), stop=(kt == 1))
                        r0v = nt * 8
                        nc.scalar.activation(
                            out=vpad[:, mt, 2 + r0v:2 + r0v + 8, 2:2 + W],
                            in_=ps[:].rearrange("p (r c) -> p r c", c=W),
                            func=ACTF.Identity, bias=b1cols[mt][:, 0:1],
                            scale=1.0)

                taps = [(eh, ew) for eh in range(-2, 3) for ew in range(-2, 3)]
                vblock_sched = {0: [0, 1, 2], 1: [3, 4], 2: [5, 6], 3: [7]}

                om_tiles = {}

                def emit_om(c):
                    # om projection for chunk c; emitted ahead of the tap
                    # quarters so the next A-build never waits on the PE
                    # queue behind the tap matmuls
                    om_c = ombuf.tile([128, cpt, OMW], bf16, name="om16")
                    for pi in range(cpt):
                        pt = c * cpt + pi
                        ps = psA.tile([128, OMW], f32, name="omm")
                        for kt in range(2):
                            nc.tensor.matmul(
                                ps, lhsT=xs[kt][:, pt * 128:(pt + 1) * 128],
                                rhs=wtoms[kt][:, :], start=(kt == 0), stop=False)
                        nc.tensor.matmul(ps, lhsT=ones[0:1, 0:128],
                                         rhs=bom1[0:1, :], start=False, stop=True)
                        nc.scalar.activation(out=om_c[:, pi, :], in_=ps,
                                             func=ACTF.Copy)
                    om_tiles[c] = om_c

                emit_om(0)
                emit_om(1)

                def emit_abuild(chk):
                    om_c = om_tiles.pop(chk)

                    oh = om_c[:, :, 0:144]
                    ow = om_c[:, :, 144:288]
                    mbf = om_c[:, :, 288:432]

                    th3 = tbuf.tile([128, 3, cpt, 144], bf16, name="th3")
                    tw3 = tbuf.tile([128, 3, cpt, 144], bf16, name="tw3")
                    th = [th3[:, i] for i in range(3)]
                    tw = [tw3[:, i] for i in range(3)]

                    # tents (bf16): index 0,1,2 <-> i=-1,0,+1; slot1 holds
                    # NEGATED t(0) = |o|-1; sign fixed at scatter time.
                    nc.scalar.activation(out=th[2], in_=oh, func=ACTF.Relu)
                    nc.scalar.activation(out=tw[2], in_=ow, func=ACTF.Relu)
                    nc.scalar.activation(out=th[0], in_=oh, func=ACTF.Relu, scale=-1.0)
                    nc.scalar.activation(out=tw[0], in_=ow, func=ACTF.Relu, scale=-1.0)
                    nc.vector.tensor_tensor(out=th[1], in0=th[2], in1=th[0], op=ALU.add)
                    nc.vector.tensor_scalar(out=th[1], in0=th[1], scalar1=-1.0,
                                            scalar2=None, op0=ALU.add)
                    nc.vector.tensor_tensor(out=tw[1], in0=tw[2], in1=tw[0], op=ALU.add)
                    nc.vector.tensor_scalar(out=tw[1], in0=tw[1], scalar1=-1.0,
                                            scalar2=None, op0=ALU.add)
                    # th *= mask: one op over all 3 tent slots, mask
                    # broadcast via a stride-0 dim
                    t3_ap = bass.AP(
                        th3[:, :, :, :].tensor, th3[:, :, :, :].offset,
                        [[3 * cpt * 144, 128], [cpt * 144, 3], [144, cpt], [1, 144]])
                    m_ap = om_c[:, :, :]
                    mb_ap = bass.AP(
                        m_ap.tensor, m_ap.offset + 288,
                        [[cpt * OMW, 128], [0, 3], [OMW, cpt], [1, 144]])
                    nc.vector.tensor_tensor(out=t3_ap, in0=t3_ap, in1=mb_ap,
                                            op=ALU.mult)

                    # A^T chunk [128, cpt, (TPAD t, 16 g)]
                    at = atbuf.tile([128, cpt, TPAD * 16], bf16, name="at")
                    if chk < 3:
                        nc.gpsimd.memset(at, 0.0)
                    else:
                        at32 = at[:].rearrange("p a b -> p (a b)").bitcast(
                            mybir.dt.uint32)
                        nc.scalar.mul(at32, at32, 0.0)
                    prod3 = tbuf.tile([128, 3, cpt, 144], bf16, name="prod3")
                    p3_ap = bass.AP(
                        prod3[:, :, :, :].tensor, prod3[:, :, :, :].offset,
                        [[3 * cpt * 144, 128], [cpt * 144, 3], [144, cpt], [1, 144]])
                    tw_all = bass.AP(
                        tw3[:, :, :, :].tensor, tw3[:, :, :, :].offset,
                        [[3 * cpt * 144, 128], [cpt * 144, 3], [144, cpt], [1, 144]])
                    for i in range(3):
                        # prod3[:, j] = th[i] * tw[j] for all j in one op
                        # (th[i] broadcast over j via stride-0)
                        thb = bass.AP(
                            th3[:, :, :, :].tensor,
                            th3[:, :, :, :].offset + i * cpt * 144,
                            [[3 * cpt * 144, 128], [0, 3], [144, cpt], [1, 144]])
                        nc.vector.tensor_tensor(out=p3_ap, in0=thb, in1=tw_all,
                                                op=ALU.mult)
                        for j in range(3):
                            a_ap = at[:, :, :]
                            o_ap = bass.AP(
                                a_ap.tensor,
                                a_ap.offset + (i * 5 + j) * 16,
                                [[cpt * TPAD * 16, 128], [TPAD * 16, cpt],
                                 [5 * 16, 3], [1, 48]])
                            i_ap = bass.AP(
                                prod3[:, :, :, :].tensor,
                                prod3[:, :, :, :].offset + j * cpt * 144,
                                [[3 * cpt * 144, 128], [144, cpt], [48, 3], [1, 48]])
                            sop = ALU.subtract if (i == 1) != (j == 1) else ALU.add
                            nc.vector.tensor_tensor(out=o_ap, in0=o_ap,
                                                    in1=i_ap, op=sop)

                    # transpose A^T -> A tiles [(t8, g16), pix]
                    for tb in range(4):
                        tps = trps.tile([128, 512], bf16, name="tr")
                        for s in range(4):
                            nc.tensor.transpose(
                                tps[:, s * 128:(s + 1) * 128],
                                at[:, s, tb * 128:(tb + 1) * 128],
                                idn[:, :])
                        col = chk * cpt * 128
                        nc.scalar.activation(
                            out=atile[tb][:, col:col + 512], in_=tps,
                            func=ACTF.Copy)

                emit_abuild(0)
                emit_abuild(1)

                for qq in range(4):
                    r0 = qq * 16
                    for ntv in vblock_sched[qq]:
                        emit_vblock(ntv)
                    c2 = 2 * qq + 2
                    if c2 < n_chunk:
                        emit_om(c2)
                        emit_om(c2 + 1)
                        emit_abuild(c2)
                        emit_abuild(c2 + 1)
                    with tc.tile_pool(name=f"ups{qq}", bufs=1,
                                      space="PSUM") as upsp:
                        ups = [upsp.tile([128, QPIX], f32,
                                         name=f"ups_{qq}_{v}")
                               for v in range(2)]
                        pool_share = (15, 15, 14, 10)[qq]
                        ntap = len(taps)
                        first_t, last_t = 0, ntap - 1
                        for t in range(ntap):
                            eh, ew = taps[t]
                            slot = (eh + 2) * 5 + (ew + 2)
                            tb, ts = slot // 8, slot % 8
                            mul = 21 if qq == 3 else qq * 3
                            on_pool = ((t * 5 + mul) % ntap) < pool_share
                            abc = (abcq if on_pool else abcp).tile(
                                [128, QPIX], bf16, name="abc")
                            a_ap = atile[tb][:, :]
                            sap = bass.AP(
                                a_ap.tensor,
                                a_ap.offset + ts * 16 * HW + qq * QPIX,
                                [[HW, 16], [0, 8], [1, QPIX]])
                            nc.sync.dma_start(out=abc, in_=sap)
                            # one product covers both channel tiles: the
                            # A map is broadcast over vt via a stride-0
                            # dim; vpad holds vt as a real dim
                            ab_ap = abc[:, :]
                            abc4 = bass.AP(
                                ab_ap.tensor, ab_ap.offset,
                                [[QPIX, 128], [0, 2], [W, 16], [1, W]])
                            vp_ap = vpad[:, :, :, :]
                            win4 = bass.AP(
                                vp_ap.tensor,
                                vp_ap.offset + (2 + r0 + eh) * PW + 2 + ew,
                                [[2 * PH * PW, 128], [PH * PW, 2],
                                 [PW, 16], [1, W]])
                            pr = (prodq if on_pool else prodp).tile(
                                [128, 2, QPIX], bf16, name="tp")
                            pr_ap = pr[:, :, :]
                            pr4 = bass.AP(
                                pr_ap.tensor, pr_ap.offset,
                                [[2 * QPIX, 128], [QPIX, 2], [W, 16], [1, W]])
                            if on_pool:
                                # gpsimd supports only TensorTensor for
                                # products (3D APs); Pool takes vt0 and DVE
                                # takes vt1 so the split-tap costs balance
                                abc3 = abc[:].rearrange("p (h w) -> p h w", w=W)
                                for vt, eng in ((0, nc.gpsimd), (1, nc.vector)):
                                    win3 = bass.AP(
                                        vp_ap.tensor,
                                        vp_ap.offset + vt * PH * PW
                                        + (2 + r0 + eh) * PW + 2 + ew,
                                        [[2 * PH * PW, 128], [PW, 16], [1, W]])
                                    pr3 = bass.AP(
                                        pr_ap.tensor, pr_ap.offset + vt * QPIX,
                                        [[2 * QPIX, 128], [W, 16], [1, W]])
                                    eng.tensor_tensor(out=pr3, in0=abc3,
                                                      in1=win3, op=ALU.mult)
                            else:
                                nc.vector.tensor_tensor(
                                    out=pr4, in0=abc4, in1=win4, op=ALU.mult)
                            for vt in range(2):
                                for nb in range(2):
                                    nc.tensor.matmul(
                                        ups[vt][:, nb * 512:(nb + 1) * 512],
                                        lhsT=idn[:, :],
                                        rhs=pr[:, vt, nb * 512:(nb + 1) * 512],
                                        start=(t == first_t),
                                        stop=(t == last_t))
                        if qq >= 2:
                            # final quarter: split the two PSUM->SBUF copies
                            # across DVE+ACT to shorten the drain chain (DVE
                            # is idle once the last products are done)
                            nc.vector.tensor_copy(
                                out=usb[0][:, qq * QPIX:(qq + 1) * QPIX],
                                in_=ups[0])
                            nc.scalar.activation(
                                out=usb[1][:, qq * QPIX:(qq + 1) * QPIX],
                                in_=ups[1], func=ACTF.Copy)
                        else:
                            for vt in range(2):
                                nc.scalar.activation(
                                    out=usb[vt][:, qq * QPIX:(qq + 1) * QPIX],
                                    in_=ups[vt], func=ACTF.Copy)

                    # cv2 + BN + SiLU for this quarter's pixel columns
                    with tc.tile_pool(name=f"cvps{qq}", bufs=2,
                                      space="PSUM") as cvps, \
                         tc.tile_pool(name=f"ysb{qq}", bufs=2) as ysbp:
                        for mt in range(2):
                            ps2 = cvps.tile([128, 1024], f32, name="cv2ps")
                            for lnt, nt in enumerate((2 * qq, 2 * qq + 1)):
                                for kt in range(2):
                                    nc.tensor.matmul(
                                        ps2[:, lnt * 512:(lnt + 1) * 512],
                                        lhsT=wt2s[kt][:, mt * 128:(mt + 1) * 128],
                                        rhs=usb[kt][:, nt * 512:(nt + 1) * 512],
                                        start=(kt == 0), stop=(kt == 1))
                            ysb = ysbp.tile([128, 1024], f32, name="ysb")
                            nc.scalar.activation(
                                out=ysb, in_=ps2, func=ACTF.Silu,
                                bias=b2s[mt][:, 0:1], scale=1.0)
                            nc.sync.dma_start(
                                out=y_d[mt * 128:(mt + 1) * 128,
                                        qq * 1024:(qq + 1) * 1024],
                                in_=ysb)

    _split_multiwait(nc, mybir)
    return nc


def _prepare(inputs):
    x = np.ascontiguousarray(np.asarray(inputs["x"], np.float32))
    w_cv1 = np.asarray(inputs["w_cv1"], np.float32)
    b_cv1 = np.asarray(inputs["b_cv1"], np.float32)
    w_off = np.asarray(inputs["w_off"], np.float32)
    b_off = np.asarray(inputs["b_off"], np.float32)
    w_cv2 = np.asarray(inputs["w_cv2"], np.float32)
    bn_g = np.asarray(inputs["bn_gamma"], np.float32)
    bn_b = np.asarray(inputs["bn_beta"], np.float32)
    bn_m = np.asarray(inputs["bn_mean"], np.float32)
    bn_v = np.asarray(inputs["bn_var"], np.float32)

    perm_v = _v_perm()
    W1p = w_cv1[perm_v, :]
    b1p = b_cv1[perm_v]

    Wom = w_off @ w_cv1
    bom = w_off @ b_cv1 + b_off
    omp = _om_perm()
    Wom_big = np.zeros((OMW, C1), np.float32)
    Wom_big[:432] = Wom[omp]
    bom_big = np.zeros((OMW,), np.float32)
    bom_big[:432] = bom[omp]

    s = bn_g / np.sqrt(bn_v + BN_EPS)
    W2s = w_cv2 * s[:, None]
    b2f = bn_b - bn_m * s
    W2p = W2s[:, perm_v]

    shared = dict(
        wt1=np.ascontiguousarray(W1p.T),
        wtom=np.ascontiguousarray(Wom_big.T),
        wt2=np.ascontiguousarray(W2p.T).astype(ml_dtypes.bfloat16),
        b1c=np.ascontiguousarray(b1p[:, None]),
        b2=np.ascontiguousarray(b2f[:, None]),
        bom=np.ascontiguousarray(bom_big[None, :]),
        idn=np.eye(128, dtype=ml_dtypes.bfloat16),
        onesrow=np.ones((1, 128), np.float32),
    )
    in_maps = []
    for b in range(B):
        m = dict(shared)
        m["x"] = np.ascontiguousarray(x[b].reshape(C1, HW))
        in_maps.append(m)
    return in_maps


def kernel(**inputs):
    from concourse.bass_utils import run_bass_kernel_spmd

    if "nc" not in _cache:
        _cache["nc"] = _build_nc()
    nc = _cache["nc"]
    in_maps = _prepare(inputs)
    res = run_bass_kernel_spmd(nc, in_maps, core_ids=list(range(B)))
    out = np.stack([r["y"].reshape(C2, H, W) for r in res.results])
    return out.astype(np.float32)


if __name__ == "__main__":
    rng = np.random.default_rng(0)
    demo = dict(
        x=rng.standard_normal((B, C1, H, W)).astype(np.float32),
        w_cv1=rng.standard_normal((C, C1)).astype(np.float32) / 16,
        b_cv1=(rng.standard_normal((C,)) * 0.1).astype(np.float32),
        w_off=(rng.standard_normal((G * 3 * K, C)) * 0.01).astype(np.float32),
        b_off=(rng.standard_normal((G * 3 * K,)) * 0.01).astype(np.float32),
        w_cv2=rng.standard_normal((C2, C)).astype(np.float32) / 16,
        bn_gamma=rng.uniform(0.5, 1.5, (C2,)).astype(np.float32),
        bn_beta=(rng.standard_normal((C2,)) * 0.1).astype(np.float32),
        bn_mean=(rng.standard_normal((C2,)) * 0.1).astype(np.float32),
        bn_var=rng.uniform(0.5, 1.5, (C2,)).astype(np.float32),
    )
    y = kernel(**demo)
    print("kernel ran, output", y.shape, y.dtype)



# revision 1
# speedup vs baseline: 1.0572x; 1.0572x over previous
"""DCNv4 block (cv1 1x1 -> offset/mask proj -> deformable bilinear sampling
-> cv2 1x1 -> BN -> SiLU) as a Bass/Tile kernel for Trainium2.

Strategy
--------
Data-parallel over batch: each of the 8 NeuronCores processes one image.

The deformable sampling is reformulated gather-free: with |off| < 1 the
bilinear sample of kernel point k at (h+kh+off_h, w+kw+off_w) equals
  sum_{i,j in {-1,0,1}} tent(off_h - i) * tent(off_w - j) * V[h+kh+i, w+kw+j]
with tent(t) = max(0, 1-|t|).  Merging all 9 kernel points over absolute
displacements e=(eh,ew) in [-2,2]^2 gives 25 "taps":
  out[p,g,:] = sum_e A_e[p,g] * Vpad[p+e, g, :]
  A_e[p,g]   = sum_k mask_k * tent(off_h - (eh-kh)) * tent(off_w - (ew-kw))
Out-of-image corners are handled exactly by zero-padding Vpad (the reference
drops those corners).

Engine mapping / schedule:
 - PE: cv1 / om / cv2 matmuls (f32r), A^T transposes, and the 25-term tap
   accumulation as identity-weight matmuls into PSUM.  om/cv1 matmuls are
   emitted just-in-time (one quarter ahead) so the deep PE exec queue always
   holds satisfied-dep work and the p-state stays ramped.
 - DVE: tent slot-1 builds, mask multiply and the 9 tent products (merged
   over i/j via stride-0 broadcast dims), 9 scatter-adds into A^T, and most
   tap products as single [128, 2, 1024] ops covering both channel tiles
   (the A map broadcast over vt with a stride-0 dim, vpad holding vt as a
   real dim).
 - GPSIMD: A^T zeroing for early chunks, pad-ring memsets, and for a
   balanced share of taps the vt0 product (TensorTensor; the cost of DVE's
   remaining vt1 single then balances the two engines).
 - ACT: om bf16 copies (the mask slice doubles as the bf16 mask, no
   separate cast), tent relus, cv1-bias copies (bias rides Identity), A^T
   zeroing late chunks, atile/usb copies, BN+SiLU epilogue.
 - Schedule: A-build for chunks 2q+2, 2q+3 is emitted BEFORE tap quarter q
   (software pipelining), so neither DVE nor ACT ever wait behind a
   quarter's tap/idn stream; separate tile pools for Pool- vs DVE-assigned
   taps prevent buffer-rotation convoys.
"""

import sys
import numpy as np

if "/opt/trn_rl_repo" not in sys.path:
    sys.path.insert(0, "/opt/trn_rl_repo")

import ml_dtypes

B, C1, C2, H, W = 8, 256, 256, 64, 64
C = 256
G = 16
Cg = 16
K = 9
HW = H * W           # 4096
PW = W + 4           # 68
PH = H + 4
BN_EPS = 1e-5
TPAD = 32            # taps padded to 32 so (t, g) blocks are 128-aligned
OMW = 448            # om channels padded 432 -> 448

_cache = {}


def _v_perm():
    perm = []
    for vt in range(2):
        for j in range(128):
            g = j // 8
            c = vt * 8 + (j % 8)
            perm.append(g * Cg + c)
    return np.array(perm, np.int64)


def _om_perm():
    rows = np.zeros(432, np.int64)
    for r in range(144):
        k, g = r // 16, r % 16
        rows[r] = g * 27 + 2 * k            # off_h (dh)
        rows[144 + r] = g * 27 + 2 * k + 1  # off_w (dw)
        rows[288 + r] = g * 27 + 18 + k     # mask
    return rows


def _split_multiwait(nc, mybir, max_waits=1):
    """walrus in this container rejects >1 sem wait on one instruction;
    split extras onto preceding same-engine NoOps (equivalent ordering)."""
    for f in nc.m.functions:
        for bb in f.blocks:
            out = []
            for inst in bb.instructions:
                si = inst.sync_info
                if si is not None and len(si.on_wait) > max_waits:
                    waits = list(si.on_wait)
                    for w in waits[:-max_waits]:
                        nop = mybir.InstNoOp(
                            name=f"I-nopw{nc.next_id()}", ins=[], outs=[])
                        nop.engine = inst.engine
                        nop.sync_info = mybir.SyncInfo(on_wait=[w], on_update=[])
                        nc.register_instruction(nop)
                        out.append(nop)
                    si.on_wait = waits[-max_waits:]
                out.append(inst)
            bb.instructions = out


def _build_nc(phase=99):
    import concourse.bass as bass
    import concourse.mybir as mybir
    import concourse.tile as tile

    f32 = mybir.dt.float32
    f32r = mybir.dt.float32r
    bf16 = mybir.dt.bfloat16
    ALU = mybir.AluOpType
    ACTF = mybir.ActivationFunctionType

    nc = bass.Bass()

    x_d = nc.dram_tensor("x", [C1, HW], f32r, kind="ExternalInput")
    wt1_d = nc.dram_tensor("wt1", [C1, 256], f32r, kind="ExternalInput")
    wtom_d = nc.dram_tensor("wtom", [C1, OMW], f32r, kind="ExternalInput")
    wt2_d = nc.dram_tensor("wt2", [C, C2], bf16, kind="ExternalInput")
    b1c_d = nc.dram_tensor("b1c", [C, 1], f32, kind="ExternalInput")
    b2_d = nc.dram_tensor("b2", [C2, 1], f32, kind="ExternalInput")
    bom_d = nc.dram_tensor("bom", [1, OMW], f32r, kind="ExternalInput")
    idn_d = nc.dram_tensor("idn", [128, 128], bf16, kind="ExternalInput")
    ones_d = nc.dram_tensor("onesrow", [1, 128], f32r, kind="ExternalInput")
    y_d = nc.dram_tensor("y", [C2, HW], f32, kind="ExternalOutput")

    cpt = 4
    n_chunk = 8
    QPIX = 1024
    EHS = (-2, -1, 0, 1, 2)

    with tile.TileContext(nc) as tc:
        with tc.tile_pool(name="persist", bufs=1) as persist:

            # ---- persistent tiles ----
            wt1s = [persist.tile([128, 256], f32r, name=f"wt1_{i}") for i in range(2)]
            wtoms = [persist.tile([128, OMW], f32r, name=f"wtom_{i}") for i in range(2)]
            wt2s = [persist.tile([128, 256], bf16, name=f"wt2_{i}") for i in range(2)]
            bom1 = persist.tile([1, OMW], f32r, name="bom1")
            b1cols = [persist.tile([128, 1], f32, name=f"b1c_{i}") for i in range(2)]
            b2s = [persist.tile([128, 1], f32, name=f"b2_{i}") for i in range(2)]
            ones = persist.tile([1, 128], f32r, name="ones")
            idn = persist.tile([128, 128], bf16, name="idn")
            vpad = persist.tile([128, 2, PH, PW], bf16, name="vpad")
            atile = [persist.tile([128, HW], bf16, name=f"atile_{i}") for i in range(4)]
            usb = [persist.tile([128, HW], bf16, name=f"usb_{v}") for v in range(2)]

            # zero the pad ring of Vpad (interior written by cv1)
            for vt in range(2):
                nc.gpsimd.memset(vpad[:, vt, 0:2, :], 0.0)
                nc.gpsimd.memset(vpad[:, vt, PH - 2:PH, :], 0.0)
                nc.gpsimd.memset(vpad[:, vt, 2:PH - 2, 0:2], 0.0)
                nc.gpsimd.memset(vpad[:, vt, 2:PH - 2, PW - 4:PW], 0.0)

            # ---- interleaved: om chunks + A-build + tap quarters.  om/cv1
            # matmuls stay just-in-time per chunk so the deep PE exec queue
            # always holds satisfied-dep work (keeps the PE p-state ramped)
            with tc.tile_pool(name="build", bufs=1) as bpool, \
                 tc.tile_pool(name="psA", bufs=2, space="PSUM") as psA, \
                 tc.tile_pool(name="ombuf", bufs=4) as ombuf, \
                 tc.tile_pool(name="tbuf", bufs=2) as tbuf, \
                 tc.tile_pool(name="atbuf", bufs=2) as atbuf, \
                 tc.tile_pool(name="trps", bufs=2, space="PSUM") as trps, \
                 tc.tile_pool(name="abcp", bufs=5) as abcp, \
                 tc.tile_pool(name="abcq", bufs=5) as abcq, \
                 tc.tile_pool(name="prodp", bufs=3) as prodp, \
                 tc.tile_pool(name="prodq", bufs=4) as prodq:

                scr = bpool.tile([128, 512], bf16, name="pewarm")
                nc.vector.memset(scr, 0.0)
                for wub in range(6):
                    wps = psA.tile([128, 512], f32, name="omm")
                    nc.tensor.matmul(wps, lhsT=scr[:, 0:128], rhs=scr[:, :],
                                     start=True, stop=True)

                xs = [bpool.tile([128, HW], f32r, name=f"xs_{i}") for i in range(2)]
                for i in range(2):
                    nc.sync.dma_start(out=xs[i][:, 0:512],
                                      in_=x_d[i * 128:(i + 1) * 128, 0:512])
                for i in range(2):
                    nc.sync.dma_start(out=wtoms[i],
                                      in_=wtom_d[i * 128:(i + 1) * 128, :])
                nc.sync.dma_start(out=bom1, in_=bom_d[:, :])
                nc.sync.dma_start(out=ones, in_=ones_d[:, :])
                for q4 in range(4):
                    lo = 512 if q4 == 0 else q4 * 1024
                    for i in range(2):
                        nc.sync.dma_start(
                            out=xs[i][:, lo:(q4 + 1) * 1024],
                            in_=x_d[i * 128:(i + 1) * 128, lo:(q4 + 1) * 1024])
                nc.sync.dma_start(out=idn, in_=idn_d[:, :])
                for i in range(2):
                    nc.sync.dma_start(out=wt1s[i], in_=wt1_d[i * 128:(i + 1) * 128, :])
                    nc.sync.dma_start(out=wt2s[i], in_=wt2_d[i * 128:(i + 1) * 128, :])
                    nc.sync.dma_start(out=b1cols[i], in_=b1c_d[i * 128:(i + 1) * 128, :])
                    nc.sync.dma_start(out=b2s[i], in_=b2_d[i * 128:(i + 1) * 128, :])

                def emit_vblock(nt):
                    # cv1 rows nt*8..nt*8+8; bias rides the activation copy
                    for mt in range(2):
                        ps = psA.tile([128, 512], f32, name="omm")
                        for kt in range(2):
                            nc.tensor.matmul(
                                ps, lhsT=wt1s[kt][:, mt * 128:(mt + 1) * 128],
                                rhs=xs[kt][:, nt * 512:(nt + 1) * 512],
                                start=(kt == 0), stop=(kt == 1))
                        r0v = nt * 8
                        nc.scalar.activation(
                            out=vpad[:, mt, 2 + r0v:2 + r0v + 8, 2:2 + W],
                            in_=ps[:].rearrange("p (r c) -> p r c", c=W),
                            func=ACTF.Identity, bias=b1cols[mt][:, 0:1],
                            scale=1.0)

                taps = [(eh, ew) for eh in range(-2, 3) for ew in range(-2, 3)]
                vblock_sched = {0: [0, 1, 2], 1: [3, 4], 2: [5, 6], 3: [7]}

                om_tiles = {}

                def emit_om(c):
                    # om projection for chunk c; emitted ahead of the tap
                    # quarters so the next A-build never waits on the PE
                    # queue behind the tap matmuls
                    om_c = ombuf.tile([128, cpt, OMW], bf16, name="om16")
                    for pi in range(cpt):
                        pt = c * cpt + pi
                        ps = psA.tile([128, OMW], f32, name="omm")
                        for kt in range(2):
                            nc.tensor.matmul(
                                ps, lhsT=xs[kt][:, pt * 128:(pt + 1) * 128],
                                rhs=wtoms[kt][:, :], start=(kt == 0), stop=False)
                        nc.tensor.matmul(ps, lhsT=ones[0:1, 0:128],
                                         rhs=bom1[0:1, :], start=False, stop=True)
                        nc.scalar.activation(out=om_c[:, pi, :], in_=ps,
                                             func=ACTF.Copy)
                    om_tiles[c] = om_c

                emit_om(0)
                emit_om(1)

                def emit_abuild(chk):
                    om_c = om_tiles.pop(chk)

                    oh = om_c[:, :, 0:144]
                    ow = om_c[:, :, 144:288]
                    mbf = om_c[:, :, 288:432]

                    th3 = tbuf.tile([128, 3, cpt, 144], bf16, name="th3")
                    tw3 = tbuf.tile([128, 3, cpt, 144], bf16, name="tw3")
                    th = [th3[:, i] for i in range(3)]
                    tw = [tw3[:, i] for i in range(3)]

                    # tents (bf16): index 0,1,2 <-> i=-1,0,+1; slot1 holds
                    # NEGATED t(0) = |o|-1; sign fixed at scatter time.
                    nc.scalar.activation(out=th[2], in_=oh, func=ACTF.Relu)
                    nc.scalar.activation(out=tw[2], in_=ow, func=ACTF.Relu)
                    nc.scalar.activation(out=th[0], in_=oh, func=ACTF.Relu, scale=-1.0)
                    nc.scalar.activation(out=tw[0], in_=ow, func=ACTF.Relu, scale=-1.0)
                    nc.vector.tensor_tensor(out=th[1], in0=th[2], in1=th[0], op=ALU.add)
                    nc.vector.tensor_scalar(out=th[1], in0=th[1], scalar1=-1.0,
                                            scalar2=None, op0=ALU.add)
                    nc.vector.tensor_tensor(out=tw[1], in0=tw[2], in1=tw[0], op=ALU.add)
                    nc.vector.tensor_scalar(out=tw[1], in0=tw[1], scalar1=-1.0,
                                            scalar2=None, op0=ALU.add)
                    # th *= mask: one op over all 3 tent slots, mask
                    # broadcast via a stride-0 dim
                    t3_ap = bass.AP(
                        th3[:, :, :, :].tensor, th3[:, :, :, :].offset,
                        [[3 * cpt * 144, 128], [cpt * 144, 3], [144, cpt], [1, 144]])
                    m_ap = om_c[:, :, :]
                    mb_ap = bass.AP(
                        m_ap.tensor, m_ap.offset + 288,
                        [[cpt * OMW, 128], [0, 3], [OMW, cpt], [1, 144]])
                    nc.vector.tensor_tensor(out=t3_ap, in0=t3_ap, in1=mb_ap,
                                            op=ALU.mult)

                    # A^T chunk [128, cpt, (TPAD t, 16 g)]
                    at = atbuf.tile([128, cpt, TPAD * 16], bf16, name="at")
                    if chk < 3:
                        nc.gpsimd.memset(at, 0.0)
                    else:
                        at32 = at[:].rearrange("p a b -> p (a b)").bitcast(
                            mybir.dt.uint32)
                        nc.scalar.mul(at32, at32, 0.0)
                    prod3 = tbuf.tile([128, 3, cpt, 144], bf16, name="prod3")
                    p3_ap = bass.AP(
                        prod3[:, :, :, :].tensor, prod3[:, :, :, :].offset,
                        [[3 * cpt * 144, 128], [cpt * 144, 3], [144, cpt], [1, 144]])
                    tw_all = bass.AP(
                        tw3[:, :, :, :].tensor, tw3[:, :, :, :].offset,
                        [[3 * cpt * 144, 128], [cpt * 144, 3], [144, cpt], [1, 144]])
                    for i in range(3):
                        # prod3[:, j] = th[i] * tw[j] for all j in one op
                        # (th[i] broadcast over j via stride-0)
                        thb = bass.AP(
                            th3[:, :, :, :].tensor,
                            th3[:, :, :, :].offset + i * cpt * 144,
                            [[3 * cpt * 144, 128], [0, 3], [144, cpt], [1, 144]])
                        nc.vector.tensor_tensor(out=p3_ap, in0=thb, in1=tw_all,
                                                op=ALU.mult)
                        for j in range(3):
                            a_ap = at[:, :, :]
                            o_ap = bass.AP(
                                a_ap.tensor,
                                a_ap.offset + (i * 5 + j) * 16,
                                [[cpt * TPAD * 16, 128], [TPAD * 16, cpt],
                                 [5 * 16, 3], [1, 48]])
                            i_ap = bass.AP(
                                prod3[:, :, :, :].tensor,
                                prod3[:, :, :, :].offset + j * cpt * 144,
                                [[3 * cpt * 144, 128], [144, cpt], [48, 3], [1, 48]])
                            sop = ALU.subtract if (i == 1) != (j == 1) else ALU.add
                            nc.vector.tensor_tensor(out=o_ap, in0=o_ap,
                                                    in1=i_ap, op=sop)

                    # transpose A^T -> A tiles [(t8, g16), pix]
                    for tb in range(4):
                        tps = trps.tile([128, 512], bf16, name="tr")
                        for s in range(4):
                            nc.tensor.transpose(
                                tps[:, s * 128:(s + 1) * 128],
                                at[:, s, tb * 128:(tb + 1) * 128],
                                idn[:, :])
                        col = chk * cpt * 128
                        nc.scalar.activation(
                            out=atile[tb][:, col:col + 512], in_=tps,
                            func=ACTF.Copy)

                emit_abuild(0)
                emit_abuild(1)

                for qq in range(4):
                    r0 = qq * 16
                    for ntv in vblock_sched[qq]:
                        emit_vblock(ntv)
                    c2 = 2 * qq + 2
                    if c2 < n_chunk:
                        emit_om(c2)
                        emit_om(c2 + 1)
                        emit_abuild(c2)
                        emit_abuild(c2 + 1)
                    with tc.tile_pool(name=f"ups{qq}", bufs=1,
                                      space="PSUM") as upsp:
                        ups = [upsp.tile([128, QPIX], f32,
                                         name=f"ups_{qq}_{v}")
                               for v in range(2)]
                        pool_share = (15, 15, 14, 10)[qq]
                        ntap = len(taps)
                        first_t, last_t = 0, ntap - 1
                        for t in range(ntap):
                            eh, ew = taps[t]
                            slot = (eh + 2) * 5 + (ew + 2)
                            tb, ts = slot // 8, slot % 8
                            mul = 21 if qq == 3 else qq * 3
                            on_pool = ((t * 5 + mul) % ntap) < pool_share
                            abc = (abcq if on_pool else abcp).tile(
                                [128, QPIX], bf16, name="abc")
                            a_ap = atile[tb][:, :]
                            sap = bass.AP(
                                a_ap.tensor,
                                a_ap.offset + ts * 16 * HW + qq * QPIX,
                                [[HW, 16], [0, 8], [1, QPIX]])
                            nc.sync.dma_start(out=abc, in_=sap)
                            # one product covers both channel tiles: the
                            # A map is broadcast over vt via a stride-0
                            # dim; vpad holds vt as a real dim
                            ab_ap = abc[:, :]
                            abc4 = bass.AP(
                                ab_ap.tensor, ab_ap.offset,
                                [[QPIX, 128], [0, 2], [W, 16], [1, W]])
                            vp_ap = vpad[:, :, :, :]
                            win4 = bass.AP(
                                vp_ap.tensor,
                                vp_ap.offset + (2 + r0 + eh) * PW + 2 + ew,
                                [[2 * PH * PW, 128], [PH * PW, 2],
                                 [PW, 16], [1, W]])
                            pr = (prodq if on_pool else prodp).tile(
                                [128, 2, QPIX], bf16, name="tp")
                            pr_ap = pr[:, :, :]
                            pr4 = bass.AP(
                                pr_ap.tensor, pr_ap.offset,
                                [[2 * QPIX, 128], [QPIX, 2], [W, 16], [1, W]])
                            if on_pool:
                                # gpsimd supports only TensorTensor for
                                # products (3D APs); Pool takes vt0 and DVE
                                # takes vt1 so the split-tap costs balance
                                abc3 = abc[:].rearrange("p (h w) -> p h w", w=W)
                                for vt, eng in ((0, nc.gpsimd), (1, nc.vector)):
                                    win3 = bass.AP(
                                        vp_ap.tensor,
                                        vp_ap.offset + vt * PH * PW
                                        + (2 + r0 + eh) * PW + 2 + ew,
                                        [[2 * PH * PW, 128], [PW, 16], [1, W]])
                                    pr3 = bass.AP(
                                        pr_ap.tensor, pr_ap.offset + vt * QPIX,
                                        [[2 * QPIX, 128], [W, 16], [1, W]])
                                    eng.tensor_tensor(out=pr3, in0=abc3,
                                                      in1=win3, op=ALU.mult)
                            else:
                                nc.vector.tensor_tensor(
                                    out=pr4, in0=abc4, in1=win4, op=ALU.mult)
                            for vt in range(2):
                                for nb in range(2):
                                    nc.tensor.matmul(
                                        ups[vt][:, nb * 512:(nb + 1) * 512],
                                        lhsT=idn[:, :],
                                        rhs=pr[:, vt, nb * 512:(nb + 1) * 512],
                                        start=(t == first_t),
                                        stop=(t == last_t))
                        if qq >= 2:
                            # final quarter: split the two PSUM->SBUF copies
                            # across DVE+ACT to shorten the drain chain (DVE
                            # is idle once the last products are done)
                            nc.vector.tensor_copy(
                                out=usb[0][:, qq * QPIX:(qq + 1) * QPIX],
                                in_=ups[0])
                            nc.scalar.activation(
                                out=usb[1][:, qq * QPIX:(qq + 1) * QPIX],
                                in_=ups[1], func=ACTF.Copy)
                        else:
                            for vt in range(2):
                                nc.scalar.activation(
                                    out=usb[vt][:, qq * QPIX:(qq + 1) * QPIX],
                                    in_=ups[vt], func=ACTF.Copy)

                    # cv2 + BN + SiLU for this quarter's pixel columns
                    with tc.tile_pool(name=f"cvps{qq}", bufs=2,
                                      space="PSUM") as cvps, \
                         tc.tile_pool(name=f"ysb{qq}", bufs=2) as ysbp:
                        for mt in range(2):
                            ps2 = cvps.tile([128, 1024], f32, name="cv2ps")
                            for lnt, nt in enumerate((2 * qq, 2 * qq + 1)):
                                for kt in range(2):
                                    nc.tensor.matmul(
                                        ps2[:, lnt * 512:(lnt + 1) * 512],
                                        lhsT=wt2s[kt][:, mt * 128:(mt + 1) * 128],
                                        rhs=usb[kt][:, nt * 512:(nt + 1) * 512],
                                        start=(kt == 0), stop=(kt == 1))
                            ysb = ysbp.tile([128, 1024], f32, name="ysb")
                            nc.scalar.activation(
                                out=ysb, in_=ps2, func=ACTF.Silu,
                                bias=b2s[mt][:, 0:1], scale=1.0)
                            nc.sync.dma_start(
                                out=y_d[mt * 128:(mt + 1) * 128,
                                        qq * 1024:(qq + 1) * 1024],
                                in_=ysb)

    _split_multiwait(nc, mybir)
    return nc


def _prepare(inputs):
    x = np.ascontiguousarray(np.asarray(inputs["x"], np.float32))
    w_cv1 = np.asarray(inputs["w_cv1"], np.float32)
    b_cv1 = np.asarray(inputs["b_cv1"], np.float32)
    w_off = np.asarray(inputs["w_off"], np.float32)
    b_off = np.asarray(inputs["b_off"], np.float32)
    w_cv2 = np.asarray(inputs["w_cv2"], np.float32)
    bn_g = np.asarray(inputs["bn_gamma"], np.float32)
    bn_b = np.asarray(inputs["bn_beta"], np.float32)
    bn_m = np.asarray(inputs["bn_mean"], np.float32)
    bn_v = np.asarray(inputs["bn_var"], np.float32)

    perm_v = _v_perm()
    W1p = w_cv1[perm_v, :]
    b1p = b_cv1[perm_v]

    Wom = w_off @ w_cv1
    bom = w_off @ b_cv1 + b_off
    omp = _om_perm()
    Wom_big = np.zeros((OMW, C1), np.float32)
    Wom_big[:432] = Wom[omp]
    bom_big = np.zeros((OMW,), np.float32)
    bom_big[:432] = bom[omp]

    s = bn_g / np.sqrt(bn_v + BN_EPS)
    W2s = w_cv2 * s[:, None]
    b2f = bn_b - bn_m * s
    W2p = W2s[:, perm_v]

    shared = dict(
        wt1=np.ascontiguousarray(W1p.T),
        wtom=np.ascontiguousarray(Wom_big.T),
        wt2=np.ascontiguousarray(W2p.T).astype(ml_dtypes.bfloat16),
        b1c=np.ascontiguousarray(b1p[:, None]),
        b2=np.ascontiguousarray(b2f[:, None]),
        bom=np.ascontiguousarray(bom_big[None, :]),
        idn=np.eye(128, dtype=ml_dtypes.bfloat16),
        onesrow=np.ones((1, 128), np.float32),
    )
    in_maps = []
    for b in range(B):
        m = dict(shared)
        m["x"] = np.ascontiguousarray(x[b].reshape(C1, HW))
        in_maps.append(m)
    return in_maps


def kernel(**inputs):
    from concourse.bass_utils import run_bass_kernel_spmd

    if "nc" not in _cache:
        _cache["nc"] = _build_nc()
    nc = _cache["nc"]
    in_maps = _prepare(inputs)
    res = run_bass_kernel_spmd(nc, in_maps, core_ids=list(range(B)))
    out = np.stack([r["y"].reshape(C2, H, W) for r in res.results])
    return out.astype(np.float32)


if __name__ == "__main__":
    rng = np.random.default_rng(0)
    demo = dict(
        x=rng.standard_normal((B, C1, H, W)).astype(np.float32),
        w_cv1=rng.standard_normal((C, C1)).astype(np.float32) / 16,
        b_cv1=(rng.standard_normal((C,)) * 0.1).astype(np.float32),
        w_off=(rng.standard_normal((G * 3 * K, C)) * 0.01).astype(np.float32),
        b_off=(rng.standard_normal((G * 3 * K,)) * 0.01).astype(np.float32),
        w_cv2=rng.standard_normal((C2, C)).astype(np.float32) / 16,
        bn_gamma=rng.uniform(0.5, 1.5, (C2,)).astype(np.float32),
        bn_beta=(rng.standard_normal((C2,)) * 0.1).astype(np.float32),
        bn_mean=(rng.standard_normal((C2,)) * 0.1).astype(np.float32),
        bn_var=rng.uniform(0.5, 1.5, (C2,)).astype(np.float32),
    )
    y = kernel(**demo)
    print("kernel ran, output", y.shape, y.dtype)

